# revision 13
# baseline (speedup 1.0000x reference)
"""Masked multi-query-free attention for (B=8, S=2048, E=A=256), f32.

Sharding: data-parallel over batch B across the 8 NeuronCores (one batch
element per core, no collectives).

Per-core dataflow (all on-chip after the input DMAs):
  xT[E,S] -> qT,kT ([A,S], A on partitions; bias added on evacuation)
          -> v [S, A+1] (bias via K=1 ones-row matmul; col A holds 1.0
             so the PV matmul also produces the softmax denominator)
  scores are computed TRANSPOSED: scT[sk_chunk=128p, sq] = kT.T @ qT
  attnT = exp(scT/16) * maskT  (no max subtraction needed: |scores|<~3)
  outP[sq=128p, A+1] += attnT_chunk.T @ v_chunk   (accumulate over sk)
  out = outP[:, :A] * (1 / outP[:, A])            (per-partition scale)

Matmuls use float32r (full-rate fp32 PE mode for moving dim >= 256).
"""

import sys

sys.path.insert(0, "/opt/trn_rl_repo")

import numpy as np
import ml_dtypes

B, S, E, A = 8, 2048, 256, 256
N_CORES = 8

SQBLK = 512                 # Sq rows per outer block
N_SQBLK = S // SQBLK        # 4
SQSUB = 128                 # Sq rows per PV psum tile
N_SQSUB = SQBLK // SQSUB    # 4
SKCH = 128                  # Sk rows per score chunk (psum partitions)
N_SKCH = S // SKCH          # 16
GRP = 2                     # sk chunks per scores psum tile ([128, GRP*SQBLK])
N_GRP = N_SKCH // GRP       # 8
MTILE = 4                   # sk chunks per mask sbuf tile

SCALE = 1.0 / np.sqrt(np.float32(A))


def _emit(nc, tc, ctx, tensors):
    import concourse.bass as bass
    import concourse.mybir as mybir

    f32 = mybir.dt.float32
    f32r = mybir.dt.float32r
    AF = mybir.ActivationFunctionType

    xT, maskT, Wq, bq, Wk, bk, Wv, bv, ones_row, out = tensors

    def r(ap):  # run matmuls in float32r (full-rate fp32 mode)
        if ap.dtype != f32r:
            return ap.bitcast(f32r)
        return ap

    consts = ctx.enter_context(tc.tile_pool(name="consts", bufs=1))
    big = ctx.enter_context(tc.tile_pool(name="big", bufs=1))
    mpool = ctx.enter_context(tc.tile_pool(name="mask", bufs=16))
    epool = ctx.enter_context(tc.tile_pool(name="exp", bufs=3))
    apool = ctx.enter_context(tc.tile_pool(name="attn", bufs=3))
    opool = ctx.enter_context(tc.tile_pool(name="outsb", bufs=3))
    spool = ctx.enter_context(tc.tile_pool(name="small", bufs=4))
    ps_sc = ctx.enter_context(tc.tile_pool(name="ps_sc", bufs=2, space="PSUM"))
    ps_sm = ctx.enter_context(tc.tile_pool(name="ps_sm", bufs=4, space="PSUM"))

    # ---- inputs: spread across the three DGE rings; weights first (small),
    # then x split per Sq-block so projections can start on partial data ----
    Wq_sb, Wk_sb, Wv_sb = [], [], []
    for e in range(2):
        wq = consts.tile([128, A], f32r, tag=f"wq{e}")
        nc.sync.dma_start(out=wq, in_=Wq[e])
        Wq_sb.append(wq)
    for e in range(2):
        wk = consts.tile([128, A], f32r, tag=f"wk{e}")
        nc.gpsimd.dma_start(out=wk, in_=Wk[e])
        Wk_sb.append(wk)
        wv = consts.tile([128, A + 2], f32r, tag=f"wv{e}")
        nc.gpsimd.dma_start(out=wv, in_=Wv[e])
        Wv_sb.append(wv)
    bq_sb, bk_sb = [], []
    for a in range(2):
        t = consts.tile([128, 1], f32, tag=f"bq{a}")
        nc.sync.dma_start(out=t, in_=bq[a])
        bq_sb.append(t)
        t = consts.tile([128, 1], f32, tag=f"bk{a}")
        nc.sync.dma_start(out=t, in_=bk[a])
        bk_sb.append(t)
    bv_sb = consts.tile([1, A + 2], f32r, tag="bv")
    nc.sync.dma_start(out=bv_sb, in_=bv)
    ones_sb = consts.tile([1, 128], f32r, tag="ones")
    nc.sync.dma_start(out=ones_sb, in_=ones_row)

    # xT as per-Sq-block tiles: e=0 via sync ring, e=1 via gpsimd ring
    xT_sb = [[], []]
    for e in range(2):
        for j in range(N_SQBLK):
            t = big.tile([128, SQBLK], f32r, name=f"xt{e}_{j}", tag=f"xT{e}_{j}")
            (nc.sync if e == 0 else nc.gpsimd).dma_start(
                out=t, in_=xT[e][:, bass.ts(j, SQBLK)]
            )
            xT_sb[e].append(t)

    # ---- projections ----
    # qT/kT: [A-chunk=128p, S]; psum tile per (a, Sq512), accumulate E chunks.
    qT_sb, kT_sb = [], []
    for a in range(2):
        qt = big.tile([128, S], f32r, tag=f"qT{a}")
        kt = big.tile([128, S], f32r, tag=f"kT{a}")
        qT_sb.append(qt)
        kT_sb.append(kt)
    for a in range(2):
        for W_sb, b_sb, dst in (
            (Wq_sb, bq_sb[a], qT_sb[a]),
            (Wk_sb, bk_sb[a], kT_sb[a]),
        ):
            pss = [
                ps_sm.tile([128, 512], f32, name=f"pp{a}_{id(W_sb)}_{j}", tag="ps")
                for j in range(N_SQBLK)
            ]
            for e in range(2):
                for j in range(N_SQBLK):
                    nc.tensor.matmul(
                        pss[j],
                        lhsT=r(W_sb[e][:, bass.ts(a, 128)]),
                        rhs=r(xT_sb[e][j]),
                        start=(e == 0),
                        stop=(e == 1),
                    )
            for j in range(N_SQBLK):
                nc.vector.tensor_scalar_add(dst[:, bass.ts(j, SQBLK)], pss[j], b_sb)

    # v: [Sk-chunk=128p, A+1]; bias via K=1 ones-row matmul; col A = 1.0.
    v_sb = []
    for c in range(N_SKCH):
        ps = ps_sm.tile([128, 512], f32)
        for e in range(2):
            nc.tensor.matmul(
                ps[:, : A + 2],
                lhsT=r(xT_sb[e][c // 4][:, bass.ts(c % 4, 128)]),
                rhs=r(Wv_sb[e]),
                start=(e == 0),
                stop=False,
            )
        nc.tensor.matmul(
            ps[:, : A + 2], lhsT=r(ones_sb), rhs=r(bv_sb), start=False, stop=True
        )
        vt = big.tile([128, A + 2], f32r, tag=f"v{c}")
        nc.scalar.copy(vt, ps[:, : A + 2])
        v_sb.append(vt)

    # ---- attention ----
    mask_tiles = {}
    for j in range(N_SQBLK):
        for t in range(N_SKCH // MTILE):
            mt = mpool.tile(
                [128, MTILE, SQBLK], mybir.dt.bfloat16, name=f"mask{j}_{t}", tag="mask"
            )
            eng = nc.gpsimd if j < 2 else nc.scalar
            eng.dma_start(out=mt, in_=maskT[j][:, bass.ts(t, MTILE), :])
            mask_tiles[(j, t)] = mt

    for j in range(N_SQBLK):
        js = bass.ts(j, SQBLK)
        mask_sb = [mask_tiles[(j, t)] for t in range(N_SKCH // MTILE)]

        out_ps = [
            ps_sm.tile([128, 512], f32, name=f"out_ps{j}_{s}", tag="ps")
            for s in range(N_SQSUB)
        ]

        for g in range(N_GRP):
            sc = ps_sc.tile([128, GRP * SQBLK], f32)
            for c in range(GRP):
                ch = g * GRP + c
                for a in range(2):
                    nc.tensor.matmul(
                        sc[:, bass.ts(c, SQBLK)],
                        lhsT=r(kT_sb[a][:, bass.ts(ch, 128)]),
                        rhs=r(qT_sb[a][:, js]),
                        start=(a == 0),
                        stop=(a == 1),
                    )
            ex = epool.tile([128, GRP * SQBLK], f32)
            nc.scalar.activation(ex, sc, AF.Exp, bias=0.0, scale=float(SCALE))
            at = apool.tile([128, GRP, SQBLK], f32r)
            mslice = mask_sb[g // 2][:, bass.ds((g % 2) * GRP, GRP), :]
            nc.vector.tensor_mul(at, ex.rearrange("p (c s) -> p c s", c=GRP), mslice)
            for c in range(GRP):
                ch = g * GRP + c
                for sq in range(N_SQSUB):
                    nc.tensor.matmul(
                        out_ps[sq][:, : A + 2],
                        lhsT=r(at[:, c, bass.ts(sq, SQSUB)]),
                        rhs=r(v_sb[ch]),
                        start=(ch == 0),
                        stop=(ch == N_SKCH - 1),
                    )

        for sq in range(N_SQSUB):
            rec = spool.tile([128, 1], f32)
            nc.vector.reciprocal(rec, out_ps[sq][:, A : A + 1])
            ob = opool.tile([128, A], f32)
            nc.scalar.mul(ob, out_ps[sq][:, :A], rec)
            nc.sync.dma_start(
                out=out[j * SQBLK + sq * SQSUB : j * SQBLK + (sq + 1) * SQSUB, :],
                in_=ob,
            )


def build_nc():
    from contextlib import ExitStack

    import concourse.bacc as bacc
    import concourse.tile as tile
    import concourse.mybir as mybir

    f32 = mybir.dt.float32
    f32r = mybir.dt.float32r
    bf16 = mybir.dt.bfloat16

    nc = bacc.Bacc("TRN2", target_bir_lowering=False, debug=False)
    xT = nc.dram_tensor("xT", [2, 128, S], f32r, kind="ExternalInput").ap()
    maskT = nc.dram_tensor(
        "maskT", [N_SQBLK, 128, N_SKCH, SQBLK], bf16, kind="ExternalInput"
    ).ap()
    Wq = nc.dram_tensor("Wq", [2, 128, A], f32r, kind="ExternalInput").ap()
    bq = nc.dram_tensor("bq", [2, 128, 1], f32, kind="ExternalInput").ap()
    Wk = nc.dram_tensor("Wk", [2, 128, A], f32r, kind="ExternalInput").ap()
    bk = nc.dram_tensor("bk", [2, 128, 1], f32, kind="ExternalInput").ap()
    Wv = nc.dram_tensor("Wv", [2, 128, A + 2], f32r, kind="ExternalInput").ap()
    bv = nc.dram_tensor("bv", [1, A + 2], f32r, kind="ExternalInput").ap()
    ones_row = nc.dram_tensor("ones_row", [1, 128], f32r, kind="ExternalInput").ap()
    out = nc.dram_tensor("out", [S, A], f32, kind="ExternalOutput").ap()

    tensors = (xT, maskT, Wq, bq, Wk, bk, Wv, bv, ones_row, out)
    with tile.TileContext(nc) as tc:
        with ExitStack() as ctx:
            _emit(nc, tc, ctx, tensors)
    nc.compile()
    return nc


def pack_inputs(x, mask, Wq, bq, Wk, bk, Wv, bv):
    """Host-side packing: per-core input maps (core c <- batch c)."""
    x = np.asarray(x, dtype=np.float32)
    mask = np.asarray(mask)
    # maskT[b, j, p, c, s] = mask[b, j*512+s, c*128+p], as bf16 {0.0, 1.0}
    mt = (
        mask.transpose(0, 2, 1)
        .reshape(B, N_SKCH, 128, N_SQBLK, SQBLK)
        .transpose(0, 3, 2, 1, 4)
        .astype(ml_dtypes.bfloat16)
    )
    Wq = np.ascontiguousarray(np.asarray(Wq, np.float32)).reshape(2, 128, A)
    Wk = np.ascontiguousarray(np.asarray(Wk, np.float32)).reshape(2, 128, A)
    Wv = np.ascontiguousarray(
        np.concatenate(
            [np.asarray(Wv, np.float32), np.zeros((E, 2), np.float32)], axis=1
        ).reshape(2, 128, A + 2)
    )
    bq = np.ascontiguousarray(np.asarray(bq, np.float32)).reshape(2, 128, 1)
    bk = np.ascontiguousarray(np.asarray(bk, np.float32)).reshape(2, 128, 1)
    bv = np.concatenate(
        [np.asarray(bv, np.float32).reshape(-1), np.ones(2, np.float32)]
    ).reshape(1, A + 2)
    in_maps = []
    for b in range(N_CORES):
        xb = np.ascontiguousarray(x[b].T).reshape(2, 128, S)
        in_maps.append(
            {
                "xT": xb,
                "maskT": np.ascontiguousarray(mt[b]),
                "Wq": Wq,
                "bq": bq,
                "Wk": Wk,
                "bk": bk,
                "Wv": Wv,
                "bv": bv,
                "ones_row": np.ones((1, 128), np.float32),
            }
        )
    return in_maps


_NC_CACHE = None


def _get_nc():
    global _NC_CACHE
    if _NC_CACHE is None:
        _NC_CACHE = build_nc()
    return _NC_CACHE


def kernel(x, mask, Wq, bq, Wk, bk, Wv, bv):
    from concourse.bass_utils import run_bass_kernel_spmd

    in_maps = pack_inputs(x, mask, Wq, bq, Wk, bk, Wv, bv)
    nc = _get_nc()
    res = run_bass_kernel_spmd(nc, in_maps, core_ids=list(range(N_CORES)))
    out = np.stack([res.results[c]["out"] for c in range(N_CORES)], axis=0)
    return out.astype(np.float32)


if __name__ == "__main__":
    nc = build_nc()
    n = sum(len(bb.instructions) for bb in nc.main_func.blocks)
    print("built ok; instructions:", n)


# revision 14
# speedup vs baseline: 1.1413x; 1.1413x over previous
"""Masked multi-query-free attention for (B=8, S=2048, E=A=256), f32.

Sharding: data-parallel over batch B across the 8 NeuronCores (one batch
element per core, no collectives).

Per-core dataflow (all on-chip after the input DMAs):
  xT[E,S] -> qT,kT ([A,S], A on partitions; bias added on evacuation)
          -> v [S, A+1] (bias via K=1 ones-row matmul; col A holds 1.0
             so the PV matmul also produces the softmax denominator)
  scores are computed TRANSPOSED: scT[sk_chunk=128p, sq] = kT.T @ qT
  attnT = exp(scT/16) * maskT  (no max subtraction needed: |scores|<~3)
  outP[sq=128p, A+1] += attnT_chunk.T @ v_chunk   (accumulate over sk)
  out = outP[:, :A] * (1 / outP[:, A])            (per-partition scale)

Matmuls use float32r (full-rate fp32 PE mode for moving dim >= 256).
"""

import sys

sys.path.insert(0, "/opt/trn_rl_repo")

import numpy as np
import ml_dtypes

B, S, E, A = 8, 2048, 256, 256
N_CORES = 8

SQBLK = 512                 # Sq rows per outer block
N_SQBLK = S // SQBLK        # 4
SQSUB = 128                 # Sq rows per PV psum tile
N_SQSUB = SQBLK // SQSUB    # 4
SKCH = 128                  # Sk rows per score chunk (psum partitions)
N_SKCH = S // SKCH          # 16
GRP = 2                     # sk chunks per scores psum tile ([128, GRP*SQBLK])
N_GRP = N_SKCH // GRP       # 8
MTILE = 4                   # sk chunks per mask sbuf tile

SCALE = 1.0 / np.sqrt(np.float32(A))


def _emit(nc, tc, ctx, tensors):
    import concourse.bass as bass
    import concourse.mybir as mybir

    f32 = mybir.dt.float32
    f32r = mybir.dt.float32r
    AF = mybir.ActivationFunctionType

    xT, maskT, Wq, bias_pack, Wk, Wv, row_pack, out = tensors

    def r(ap):  # run matmuls in float32r (full-rate fp32 mode)
        if ap.dtype != f32r:
            return ap.bitcast(f32r)
        return ap

    consts = ctx.enter_context(tc.tile_pool(name="consts", bufs=1))
    big = ctx.enter_context(tc.tile_pool(name="big", bufs=1))
    mpool = ctx.enter_context(tc.tile_pool(name="mask", bufs=16))
    epool = ctx.enter_context(tc.tile_pool(name="exp", bufs=3))
    apool = ctx.enter_context(tc.tile_pool(name="attn", bufs=3))
    opool = ctx.enter_context(tc.tile_pool(name="outsb", bufs=3))
    spool = ctx.enter_context(tc.tile_pool(name="small", bufs=4))
    ps_sc = ctx.enter_context(tc.tile_pool(name="ps_sc", bufs=2, space="PSUM"))
    ps_sm = ctx.enter_context(tc.tile_pool(name="ps_sm", bufs=4, space="PSUM"))

    # ---- inputs: spread across the three DGE rings; weights first (small),
    # then x split per Sq-block so projections can start on partial data ----
    Wq_sb, Wk_sb, Wv_sb = [], [], []
    for e in range(2):
        wq = consts.tile([128, A], f32r, tag=f"wq{e}")
        nc.sync.dma_start(out=wq, in_=Wq[e])
        Wq_sb.append(wq)
    for e in range(2):
        wk = consts.tile([128, A], f32r, tag=f"wk{e}")
        nc.gpsimd.dma_start(out=wk, in_=Wk[e])
        Wk_sb.append(wk)
        wv = consts.tile([128, A + 2], f32r, tag=f"wv{e}")
        nc.gpsimd.dma_start(out=wv, in_=Wv[e])
        Wv_sb.append(wv)
    bias_sb = consts.tile([128, 4], f32, tag="bias_pack")
    nc.sync.dma_start(out=bias_sb, in_=bias_pack)
    bq_sb = [bias_sb[:, 0:1], bias_sb[:, 1:2]]
    bk_sb = [bias_sb[:, 2:3], bias_sb[:, 3:4]]
    row_sb = consts.tile([1, A + 2 + 128], f32r, tag="row_pack")
    nc.sync.dma_start(out=row_sb, in_=row_pack)
    bv_sb = row_sb[:, : A + 2]
    ones_sb = row_sb[:, A + 2 :]

    # xT as per-Sq-block tiles: e=0 via sync ring, e=1 via gpsimd ring
    xT_sb = [[], []]
    for e in range(2):
        for j in range(N_SQBLK):
            t = big.tile([128, SQBLK], f32r, name=f"xt{e}_{j}", tag=f"xT{e}_{j}")
            (nc.sync if e == 0 else nc.gpsimd).dma_start(
                out=t, in_=xT[e][:, bass.ts(j, SQBLK)]
            )
            xT_sb[e].append(t)

    # ---- projections ----
    # qT/kT: [A-chunk=128p, S]; psum tile per (a, Sq512), accumulate E chunks.
    qT_sb, kT_sb = [], []
    for a in range(2):
        qt = big.tile([128, S], f32r, tag=f"qT{a}")
        kt = big.tile([128, S], f32r, tag=f"kT{a}")
        qT_sb.append(qt)
        kT_sb.append(kt)
    for a in range(2):
        for W_sb, b_sb, dst in (
            (Wq_sb, bq_sb[a], qT_sb[a]),
            (Wk_sb, bk_sb[a], kT_sb[a]),
        ):
            pss = [
                ps_sm.tile([128, 512], f32, name=f"pp{a}_{id(W_sb)}_{j}", tag="ps")
                for j in range(N_SQBLK)
            ]
            for e in range(2):
                for j in range(N_SQBLK):
                    nc.tensor.matmul(
                        pss[j],
                        lhsT=r(W_sb[e][:, bass.ts(a, 128)]),
                        rhs=r(xT_sb[e][j]),
                        start=(e == 0),
                        stop=(e == 1),
                    )
            for j in range(N_SQBLK):
                nc.vector.tensor_scalar_add(dst[:, bass.ts(j, SQBLK)], pss[j], b_sb)

    # v: [Sk-chunk=128p, A+1]; bias via K=1 ones-row matmul; col A = 1.0.
    v_sb = []
    for c in range(N_SKCH):
        ps = ps_sm.tile([128, 512], f32)
        for e in range(2):
            nc.tensor.matmul(
                ps[:, : A + 2],
                lhsT=r(xT_sb[e][c // 4][:, bass.ts(c % 4, 128)]),
                rhs=r(Wv_sb[e]),
                start=(e == 0),
                stop=False,
            )
        nc.tensor.matmul(
            ps[:, : A + 2], lhsT=r(ones_sb), rhs=r(bv_sb), start=False, stop=True
        )
        vt = big.tile([128, A + 2], f32r, tag=f"v{c}")
        nc.scalar.copy(vt, ps[:, : A + 2])
        v_sb.append(vt)

    # ---- attention ----
    mask_tiles = {}
    for j in range(N_SQBLK):
        for t in range(N_SKCH // MTILE):
            mt = mpool.tile(
                [128, MTILE, SQBLK], mybir.dt.bfloat16, name=f"mask{j}_{t}", tag="mask"
            )
            nc.gpsimd.dma_start(out=mt, in_=maskT[j][:, bass.ts(t, MTILE), :])
            mask_tiles[(j, t)] = mt

    for j in range(N_SQBLK):
        js = bass.ts(j, SQBLK)
        mask_sb = [mask_tiles[(j, t)] for t in range(N_SKCH // MTILE)]

        out_ps = [
            ps_sm.tile([128, 512], f32, name=f"out_ps{j}_{s}", tag="ps")
            for s in range(N_SQSUB)
        ]

        for g in range(N_GRP):
            sc = ps_sc.tile([128, GRP * SQBLK], f32)
            for c in range(GRP):
                ch = g * GRP + c
                for a in range(2):
                    nc.tensor.matmul(
                        sc[:, bass.ts(c, SQBLK)],
                        lhsT=r(kT_sb[a][:, bass.ts(ch, 128)]),
                        rhs=r(qT_sb[a][:, js]),
                        start=(a == 0),
                        stop=(a == 1),
                    )
            ex = epool.tile([128, GRP * SQBLK], f32)
            nc.scalar.activation(ex, sc, AF.Exp, bias=0.0, scale=float(SCALE))
            at = apool.tile([128, GRP, SQBLK], f32r)
            mslice = mask_sb[g // 2][:, bass.ds((g % 2) * GRP, GRP), :]
            nc.vector.tensor_mul(at, ex.rearrange("p (c s) -> p c s", c=GRP), mslice)
            for c in range(GRP):
                ch = g * GRP + c
                for sq in range(N_SQSUB):
                    nc.tensor.matmul(
                        out_ps[sq][:, : A + 2],
                        lhsT=r(at[:, c, bass.ts(sq, SQSUB)]),
                        rhs=r(v_sb[ch]),
                        start=(ch == 0),
                        stop=(ch == N_SKCH - 1),
                    )

        for sq in range(N_SQSUB):
            rec = spool.tile([128, 1], f32)
            nc.vector.reciprocal(rec, out_ps[sq][:, A : A + 1])
            ob = opool.tile([128, A], f32)
            nc.scalar.mul(ob, out_ps[sq][:, :A], rec)
            nc.sync.dma_start(
                out=out[j * SQBLK + sq * SQSUB : j * SQBLK + (sq + 1) * SQSUB, :],
                in_=ob,
            )


def build_nc():
    from contextlib import ExitStack

    import concourse.bacc as bacc
    import concourse.tile as tile
    import concourse.mybir as mybir

    f32 = mybir.dt.float32
    f32r = mybir.dt.float32r
    bf16 = mybir.dt.bfloat16

    nc = bacc.Bacc("TRN2", target_bir_lowering=False, debug=False)
    xT = nc.dram_tensor("xT", [2, 128, S], f32r, kind="ExternalInput").ap()
    maskT = nc.dram_tensor(
        "maskT", [N_SQBLK, 128, N_SKCH, SQBLK], bf16, kind="ExternalInput"
    ).ap()
    Wq = nc.dram_tensor("Wq", [2, 128, A], f32r, kind="ExternalInput").ap()
    Wk = nc.dram_tensor("Wk", [2, 128, A], f32r, kind="ExternalInput").ap()
    Wv = nc.dram_tensor("Wv", [2, 128, A + 2], f32r, kind="ExternalInput").ap()
    bias_pack = nc.dram_tensor("bias_pack", [128, 4], f32, kind="ExternalInput").ap()
    row_pack = nc.dram_tensor(
        "row_pack", [1, A + 2 + 128], f32r, kind="ExternalInput"
    ).ap()
    out = nc.dram_tensor("out", [S, A], f32, kind="ExternalOutput").ap()

    tensors = (xT, maskT, Wq, bias_pack, Wk, Wv, row_pack, out)
    with tile.TileContext(nc) as tc:
        with ExitStack() as ctx:
            _emit(nc, tc, ctx, tensors)
    nc.compile()
    return nc


def pack_inputs(x, mask, Wq, bq, Wk, bk, Wv, bv):
    """Host-side packing: per-core input maps (core c <- batch c)."""
    x = np.asarray(x, dtype=np.float32)
    mask = np.asarray(mask)
    # maskT[b, j, p, c, s] = mask[b, j*512+s, c*128+p], as bf16 {0.0, 1.0}
    mt = (
        mask.transpose(0, 2, 1)
        .reshape(B, N_SKCH, 128, N_SQBLK, SQBLK)
        .transpose(0, 3, 2, 1, 4)
        .astype(ml_dtypes.bfloat16)
    )
    Wq = np.ascontiguousarray(np.asarray(Wq, np.float32)).reshape(2, 128, A)
    Wk = np.ascontiguousarray(np.asarray(Wk, np.float32)).reshape(2, 128, A)
    Wv = np.ascontiguousarray(
        np.concatenate(
            [np.asarray(Wv, np.float32), np.zeros((E, 2), np.float32)], axis=1
        ).reshape(2, 128, A + 2)
    )
    bq = np.asarray(bq, np.float32).reshape(2, 128)
    bk = np.asarray(bk, np.float32).reshape(2, 128)
    bias_pack = np.ascontiguousarray(
        np.stack([bq[0], bq[1], bk[0], bk[1]], axis=1)
    )
    row_pack = np.concatenate(
        [
            np.asarray(bv, np.float32).reshape(-1),
            np.ones(2, np.float32),
            np.ones(128, np.float32),
        ]
    ).reshape(1, A + 2 + 128)
    in_maps = []
    for b in range(N_CORES):
        xb = np.ascontiguousarray(x[b].T).reshape(2, 128, S)
        in_maps.append(
            {
                "xT": xb,
                "maskT": np.ascontiguousarray(mt[b]),
                "Wq": Wq,
                "Wk": Wk,
                "Wv": Wv,
                "bias_pack": bias_pack,
                "row_pack": row_pack,
            }
        )
    return in_maps


_NC_CACHE = None


def _get_nc():
    global _NC_CACHE
    if _NC_CACHE is None:
        _NC_CACHE = build_nc()
    return _NC_CACHE


def kernel(x, mask, Wq, bq, Wk, bk, Wv, bv):
    from concourse.bass_utils import run_bass_kernel_spmd

    in_maps = pack_inputs(x, mask, Wq, bq, Wk, bk, Wv, bv)
    nc = _get_nc()
    res = run_bass_kernel_spmd(nc, in_maps, core_ids=list(range(N_CORES)))
    out = np.stack([res.results[c]["out"] for c in range(N_CORES)], axis=0)
    return out.astype(np.float32)


if __name__ == "__main__":
    nc = build_nc()
    n = sum(len(bb.instructions) for bb in nc.main_func.blocks)
    print("built ok; instructions:", n)
